# revision 10
# baseline (speedup 1.0000x reference)
"""Trainium2 Bass kernel for segment-reduce classifier.

Reference computation:
    local = relu(x @ Wloc.T)            # [L, 128]
    feats = local.reshape(-1, 30, 128).mean(1)   # [L/30, 128]
    out   = feats @ W.T                 # [L/30, 10]

Strategy v2 (8 NeuronCores, data-parallel on rows):
  - x host-cast to fp8 e3m4 (halves DMA vs fp16; Wloc/W stay fp16 since
    weight quantization error is systematic and does not pool-average out).
    Per core xt [128, 75000]: partitions 0-63 = rows[:75000].T ("A"),
    partitions 64-127 = rows[75000:].T ("B").
  - mm1: two concurrent K=64 row-quadrant streams (tile_position rows 0/64),
    fp16 Wloc.T weights x fp8 moving operand, 512-col matmuls into
    [128, 1536] PSUM tiles (3 banks).
  - relu PSUM fp32 -> SBUF fp16 in clean unit-stride [128, 1536]
    instructions, alternating ScalarE(5) : VectorE(4) - both engines at
    1 elem/lane/cycle on fp32 PSUM reads; this pass is the roofline.
    rl keeps natural row order (no scatter) - the per-j reorder lives in
    mm2's strided access pattern, which is free on the PE.
  - mm2 (mean-pool + classifier fused): per 256-segment tile, 30
    accumulating matmuls (rhs = rl columns at stride 30, offset j) into a
    PSUM accumulator [10, 256] at column-group 32*(tile%4). Pooling is
    free PSUM accumulation; 1/30 folded into W on host. mm2 js are
    interleaved into the emission stream one tile deferred so the in-order
    PE queue alternates mm1/mm2 instead of stalling the relu engines.
  - output DMA'd straight from PSUM accumulators (no engine copy);
    host reassembles [10, 5120] per core.
"""

import numpy as np
import ml_dtypes

import concourse.bacc as bacc
import concourse.bass as bass
import concourse.tile as tile
from concourse import mybir
from concourse.bass_utils import run_bass_kernel_spmd

# Problem constants (hardcoded per harness contract)
L, D_IN, D_ENC, C, J = 1200000, 64, 128, 10, 30
N_CORES = 8
R = L // N_CORES          # rows per core = 150000
HALF = R // 2             # 75000 rows per quadrant stream
SEG_HALF = HALF // J      # 2500 segments per stream
TFS = [7680] * 9 + [5880]  # rows per stream-tile (256 / 196 segments)
PP = 1536                  # PSUM chunk per relu instruction (3 banks)
MM = 512                   # matmul output columns (1 PSUM bank)
OUTW = 5120                # out cols: A at [0,2500), B at [2560,5060)
B_OFF = 2560

_CACHE = {}


def _build_kernel():
    nc = bacc.Bacc("TRN2", target_bir_lowering=False, debug=False,
                   num_devices=N_CORES)
    f32, f16, f8 = mybir.dt.float32, mybir.dt.float16, mybir.dt.float8e3

    xt_d = nc.dram_tensor("xt", [128, HALF], f8, kind="ExternalInput")
    w1_d = nc.dram_tensor("w1", [128, D_ENC], f16, kind="ExternalInput")
    w2_d = nc.dram_tensor("w2", [128, C], f16, kind="ExternalInput")
    # 5 quads of 4 stream-tiles; quad q at cols [256q, 256q+256), group g
    # of the quad at partitions [32g, 32g+10)
    out_d = nc.dram_tensor("out", [128, 1280], f32, kind="ExternalOutput")

    with tile.TileContext(nc) as tc:
        with (
            tc.tile_pool(name="consts", bufs=1) as consts,
            tc.tile_pool(name="xin", bufs=3) as xin,
            tc.tile_pool(name="rlp", bufs=4) as rlp,
            tc.tile_pool(name="stgp", bufs=2) as stgp,
            tc.tile_pool(name="ppp", bufs=2, space="PSUM") as ppp,
            tc.tile_pool(name="accp", bufs=2, space="PSUM") as accp,
        ):
            w1 = consts.tile([128, D_ENC], f16)
            nc.sync.dma_start(w1[:], w1_d[:])
            w2 = consts.tile([128, C], f16)
            nc.sync.dma_start(w2[:], w2_d[:])

            pp_i = 0       # global relu-chunk counter (engine pattern)
            st = 0         # global stream-tile counter (acc column group)
            col0 = 0
            pend = []      # deferred mm2 emitters, one closure per j-matmul
            quad = {}      # shared PSUM accumulator across 4 stream-tiles

            def make_mm2(rl_t, tf, g, st_idx):
                gt = tf // J
                rlv = rl_t[:, 0:tf].rearrange("p (s j) -> p s j", j=J)

                def emit(j):
                    if j == 0 and g == 0:
                        quad["acc"] = accp.tile([128, 256], f32, tag="acc",
                                                name="acc")
                    acc = quad["acc"]
                    # 4 interleaved accumulation groups share one bank:
                    # only the very first matmul clears the bank's
                    # has_written bits; groups g>0 overwrite-where-clear
                    # on their first j.
                    nc.tensor.matmul(acc[32 * g:32 * g + C, 0:gt], w2[:],
                                     rlv[:, :, j],
                                     start=(j == 0 and g == 0),
                                     stop=(j == J - 1 and g == 3),
                                     tile_position=(0, 32 * g),
                                     skip_group_check=True)
                    if j == J - 1 and g == 3:
                        q = st_idx // 4
                        stg = stgp.tile([128, 256], f32, tag="stg",
                                        name="stg")
                        if q % 2 == 0:
                            nc.scalar.copy(stg[:], acc[:])
                        else:
                            nc.vector.tensor_scalar_add(stg[:], acc[:], 0.0)
                        nc.sync.dma_start(out_d[:, 256 * q:256 * (q + 1)],
                                          stg[:])
                return [lambda j=j: emit(j) for j in range(J)]

            for t, tf in enumerate(TFS):
                xt = xin.tile([128, 7680], f8, tag="xt")
                nc.sync.dma_start(xt[:, 0:tf], xt_d[:, col0:col0 + tf])

                rls = []
                for s in (0, 1):
                    rls.append(rlp.tile([128, 7680], f16, tag=f"rl{s}",
                                        name=f"rl{s}_{t}"))

                n_pp = (tf + PP - 1) // PP
                n_slots = n_pp * 2
                # mm2 emission cadence: spread pending js over the slots
                per_slot = (len(pend) + n_slots - 1) // n_slots if pend else 0

                off = 0
                for ci in range(n_pp):
                    n = min(PP, tf - off)
                    for s in (0, 1):
                        # deferred mm2 work first: PE runs these while
                        # waiting on pp buffers
                        for _ in range(per_slot):
                            if pend:
                                pend.pop(0)()
                        ppt = ppp.tile([128, PP], f32, tag="pp", name="pp")
                        for k0 in range(0, n, MM):
                            m = min(MM, n - k0)
                            nc.tensor.matmul(
                                ppt[:, k0:k0 + m],
                                w1[64 * s:64 * s + 64, :],
                                xt[64 * s:64 * s + 64, off + k0:off + k0 + m],
                                tile_position=(64 * s, 0))
                        dst = rls[s][:, off:off + n]
                        src = ppt[:, 0:n]
                        if (pp_i % 9) % 2 == 0:
                            nc.scalar.activation(
                                dst, src, mybir.ActivationFunctionType.Relu)
                        else:
                            nc.vector.tensor_scalar_max(dst, src, 0.0)
                        pp_i += 1
                    off += n

                while pend:
                    pend.pop(0)()

                for s in (0, 1):
                    g = st % 4
                    pend.extend(make_mm2(rls[s], tf, g, st))
                    st += 1
                col0 += tf

            while pend:
                pend.pop(0)()

    nc.compile()
    return nc


def kernel(x: np.ndarray, Wloc: np.ndarray, W: np.ndarray) -> np.ndarray:
    if "nc" not in _CACHE:
        _CACHE["nc"] = _build_kernel()
    nc = _CACHE["nc"]

    x = np.asarray(x, dtype=np.float32)
    # pack per-core transposed fp8e3 inputs: [8, 128, HALF]
    xp = x.reshape(N_CORES, 2, HALF, D_IN).transpose(0, 1, 3, 2)
    xp = np.ascontiguousarray(xp).astype(ml_dtypes.float8_e3m4)
    xp = xp.reshape(N_CORES, 128, HALF)

    w1 = np.ascontiguousarray(
        np.concatenate([Wloc.T, Wloc.T], axis=0), dtype=np.float16)  # [128,128]
    w2 = np.ascontiguousarray((W / float(J)).T, dtype=np.float16)    # [128,10]

    in_maps = [{"xt": xp[c], "w1": w1, "w2": w2} for c in range(N_CORES)]
    res = run_bass_kernel_spmd(nc, in_maps, core_ids=list(range(N_CORES)))
    _CACHE["exec_time_ns"] = res.exec_time_ns
    _CACHE["trace"] = res.instructions_and_trace

    out = np.empty((L // J, C), dtype=np.float32)
    segs = L // J // N_CORES          # 5000
    for c in range(N_CORES):
        oc = res.results[c]["out"]    # [128, 1280]
        base = c * segs
        for stx in range(2 * len(TFS)):
            q, g = stx // 4, stx % 4
            t, s = stx // 2, stx % 2
            gt = TFS[t] // J
            blk = oc[32 * g:32 * g + C, 256 * q:256 * q + gt]
            sb = base + s * SEG_HALF + t * 256
            out[sb:sb + gt] = blk.T
    return out


# revision 17
# speedup vs baseline: 1.9730x; 1.9730x over previous
"""Trainium2 Bass kernel for segment-reduce classifier.

Reference computation:
    local = relu(x @ Wloc.T)            # [L, 128]
    feats = local.reshape(-1, 30, 128).mean(1)   # [L/30, 128]
    out   = feats @ W.T                 # [L/30, 10]

Strategy v3 (8 NeuronCores, data-parallel on rows):
  - Per core xt [128, 75000] fp16: partitions 0-63 = rows[:75000].T ("A"),
    64-127 = rows[75000:].T ("B").  Within each 480-row block the host
    pre-permutes rows to j-major (r = j*16 + g, g = segment-in-block), so
    every on-chip access pattern is contiguous: mm1 streams contiguous
    columns, relu is a clean unit-stride copy, and mm2's per-j operand is
    contiguous 16-element runs.  (Strided matmul operands measured ~4x
    slow; fp8e3 moving operands measured 2 cyc/col - hence fp16.)
  - mm1: two concurrent K=64 row-quadrant streams (tile_position rows
    0/64), 480-col matmuls into [128, 1536] PSUM tiles (3 banks, 480
    useful cols per bank).
  - relu PSUM fp32 -> SBUF fp16 in [128, 3x480] instructions alternating
    ScalarE(5) : VectorE(4); both run ~1 elem/lane/cycle on fp32 PSUM
    reads - this pass is the roofline (~80us/core).
  - mm2 (mean-pool + classifier fused): per outer tile, 30 accumulating
    480-col matmuls (one per j; cols = 2 halves x 15 chunks x 16 segs)
    into a PSUM accumulator at column-group 32*(t%4).  Pooling is free
    PSUM accumulation; 1/30 folded into W.  Four tiles share one
    accumulator bank at disjoint partition groups (only the very first
    matmul of a quad uses start=True; later groups rely on per-element
    has_written semantics).  mm2 js are deferred two tiles and dripped
    into the emission stream between mm1 chunks so the in-order PE queue
    interleaves two column-groups without starving the relu engines.
  - Per quad: one [128, 480] copy PSUM->SBUF (alternating engines) + DMA.
"""

import numpy as np

import concourse.bacc as bacc
import concourse.bass as bass
import concourse.tile as tile
from concourse import mybir
from concourse.bass_utils import run_bass_kernel_spmd

# Problem constants (hardcoded per harness contract)
L, D_IN, D_ENC, C, J = 1200000, 64, 128, 10, 30
N_CORES = 8
R = L // N_CORES          # rows per core = 150000
HALF = R // 2             # 75000 rows per quadrant stream
SEG_HALF = HALF // J      # 2500 segments per stream
TFS = [7200] * 10 + [3000]   # rows per outer tile per stream
CH = 480                  # rows per PSUM bank chunk (16 segments)
G = CH // J               # 16 segments per chunk
N_QUADS = (len(TFS) + 3) // 4   # 3

_CACHE = {}


def _pp_layout(tf):
    """Split a tile's rows into PSUM-tile chunks: list of lists of widths."""
    pps = []
    left = tf
    while left > 0:
        w = min(left, 3 * CH)
        chunks = []
        o = 0
        while o < w:
            chunks.append(min(CH, w - o))
            o += CH
        pps.append(chunks)
        left -= w
    return pps


def _build_kernel():
    nc = bacc.Bacc("TRN2", target_bir_lowering=False, debug=False,
                   num_devices=N_CORES)
    f32, f16 = mybir.dt.float32, mybir.dt.float16

    xt_d = nc.dram_tensor("xt", [128, HALF], f16, kind="ExternalInput")
    w1_d = nc.dram_tensor("w1", [128, D_ENC], f16, kind="ExternalInput")
    w2_d = nc.dram_tensor("w2", [128, C], f16, kind="ExternalInput")
    # tile t at cols [480t, 480t + 2*gt); half h at h*gt + seg
    out_d = nc.dram_tensor("out", [C, 480 * len(TFS)], f32,
                           kind="ExternalOutput")

    with tile.TileContext(nc) as tc:
        with (
            tc.tile_pool(name="consts", bufs=1) as consts,
            tc.tile_pool(name="xin", bufs=3) as xin,
            tc.tile_pool(name="rlp", bufs=3) as rlp,
            tc.tile_pool(name="stgp", bufs=2) as stgp,
            tc.tile_pool(name="ppp", bufs=2, space="PSUM") as ppp,
            tc.tile_pool(name="accp", bufs=2, space="PSUM") as accp,
        ):
            w1 = consts.tile([128, D_ENC], f16)
            nc.sync.dma_start(w1[:], w1_d[:])
            w2 = consts.tile([128, C], f16)
            nc.sync.dma_start(w2[:], w2_d[:])

            pp_i = 0       # global relu-chunk counter (engine pattern)
            pends = []     # deques of deferred mm2 emitters, one per tile
            rr = [0]       # round-robin cursor over pends

            def make_mm2(rl_t, tf, t):
                g = t % 4
                gt = tf // J
                ncf = tf // CH           # full 480 chunks per half
                tail = tf % CH           # 120 for the last tile
                rlh = rl_t.rearrange("p (h q) -> p h q", h=2)
                main = rlh[:, :, 0:ncf * CH].rearrange(
                    "p h (c j g) -> p h c j g", j=J, g=G)
                if tail:
                    tl = rlh[:, :, ncf * CH:tf].rearrange(
                        "p h (j g) -> p h j g", j=J)
                box = {}

                def emit(j):
                    if j == 0:
                        box["acc"] = accp.tile([128, 512], f32, tag="acc",
                                               name="acc")
                    acc = box["acc"]
                    av = acc[32 * g:32 * g + C, 0:2 * gt].rearrange(
                        "p (h q) -> p h q", h=2)
                    aom = av[:, :, 0:ncf * G].rearrange(
                        "p h (c g) -> p h c g", g=G)
                    last_mm = (j == J - 1)
                    nc.tensor.matmul(aom, w2[:], main[:, :, :, j, :],
                                     start=(j == 0),
                                     stop=(last_mm and not tail),
                                     tile_position=(0, 32 * g),
                                     skip_group_check=True)
                    if tail:
                        nc.tensor.matmul(av[:, :, ncf * G:gt], w2[:],
                                         tl[:, :, j, :],
                                         start=False,
                                         stop=last_mm,
                                         tile_position=(0, 32 * g),
                                         skip_group_check=True)
                    if last_mm:
                        stg = stgp.tile([128, 480], f32, tag="stg",
                                        name="stg")
                        sl = slice(32 * g, 32 * g + C)
                        if t % 2 == 0:
                            nc.scalar.copy(stg[sl, 0:2 * gt],
                                           acc[sl, 0:2 * gt])
                        else:
                            nc.vector.tensor_scalar_add(
                                stg[sl, 0:2 * gt], acc[sl, 0:2 * gt], 0.0)
                        nc.sync.dma_start(out_d[:, 480 * t:480 * t + 2 * gt],
                                          stg[sl, 0:2 * gt])
                    return
                return emit

            def drip(n):
                for _ in range(n):
                    live = [d for d in pends if d]
                    if not live:
                        return
                    d = live[rr[0] % len(live)]
                    rr[0] += 1
                    d.pop(0)()
                while pends and not pends[0]:
                    pends.pop(0)

            col0 = 0
            for t, tf in enumerate(TFS):
                xt = xin.tile([128, 7200], f16, tag="xt", name="xt")
                nc.sync.dma_start(xt[:, 0:tf], xt_d[:, col0:col0 + tf])

                rl_t = rlp.tile([128, 14400], f16, tag="rl", name="rl")

                pps = _pp_layout(tf)
                n_slots = 2 * len(pps)
                # spread each tile's mm2 js over ~2 tile-spans so two
                # column-groups stay live on the PE simultaneously
                pend_total = sum(len(d) for d in pends)
                per_slot = -(-pend_total // (2 * n_slots)) if pend_total else 0

                off = 0
                for chunks in pps:
                    w = sum(chunks)
                    for s in (0, 1):
                        drip(per_slot)
                        ppt = ppp.tile([128, 1536], f32, tag="pp",
                                       name="pp")
                        co = 0
                        for ci, cw in enumerate(chunks):
                            nc.tensor.matmul(
                                ppt[:, 512 * ci:512 * ci + cw],
                                w1[64 * s:64 * s + 64, :],
                                xt[64 * s:64 * s + 64, off + co:off + co + cw],
                                tile_position=(64 * s, 0))
                            co += cw
                        # relu: [p, nchunks, 480] strided over banks ->
                        # contiguous rl slice
                        if len(chunks) == 3 and chunks[-1] == CH:
                            src = ppt[:, 0:1536].rearrange(
                                "p (c k) -> p c k", k=512)[:, :, 0:CH]
                        else:
                            src = ppt[:, 0:w]
                        dst = rl_t[:, 7200 * s + off:7200 * s + off + w]
                        if len(chunks) == 3 and chunks[-1] == CH:
                            dst = dst.rearrange("p (c k) -> p c k", k=CH)
                        if (pp_i % 9) % 2 == 0:
                            nc.scalar.activation(
                                dst, src, mybir.ActivationFunctionType.Relu)
                        else:
                            nc.vector.tensor_scalar_max(dst, src, 0.0)
                        pp_i += 1
                    off += w

                pends.append([])
                em = make_mm2(rl_t, tf, t)
                pends[-1].extend([lambda j=j, em=em: em(j)
                                  for j in range(J)])
                col0 += tf

            drip(10 ** 6)

    nc.compile()
    return nc


def kernel(x: np.ndarray, Wloc: np.ndarray, W: np.ndarray) -> np.ndarray:
    if "nc" not in _CACHE:
        _CACHE["nc"] = _build_kernel()
    nc = _CACHE["nc"]

    x = np.asarray(x, dtype=np.float32)
    # per-core halves, j-major permute within 480-row blocks, transpose
    xh = x.reshape(N_CORES * 2, HALF, D_IN)
    nfull = HALF // CH * CH
    a = xh[:, 0:nfull].reshape(-1, G, J, D_IN).transpose(0, 2, 1, 3)
    b = xh[:, nfull:].reshape(xh.shape[0], -1, J, D_IN).transpose(0, 2, 1, 3)
    xp = np.concatenate(
        [a.reshape(N_CORES * 2, nfull, D_IN),
         b.reshape(N_CORES * 2, HALF - nfull, D_IN)], axis=1)
    xp = xp.reshape(N_CORES, 2, HALF, D_IN).transpose(0, 1, 3, 2)
    xp = np.ascontiguousarray(xp, dtype=np.float16).reshape(N_CORES, 128, HALF)

    w1 = np.ascontiguousarray(
        np.concatenate([Wloc.T, Wloc.T], axis=0), dtype=np.float16)  # [128,128]
    w2 = np.ascontiguousarray((W / float(J)).T, dtype=np.float16)    # [128,10]

    in_maps = [{"xt": xp[c], "w1": w1, "w2": w2} for c in range(N_CORES)]
    res = run_bass_kernel_spmd(nc, in_maps, core_ids=list(range(N_CORES)))
    _CACHE["exec_time_ns"] = res.exec_time_ns
    _CACHE["trace"] = res.instructions_and_trace

    out = np.empty((L // J, C), dtype=np.float32)
    segs = L // J // N_CORES          # 5000
    segbase = np.cumsum([0] + [tf // J for tf in TFS])
    for c in range(N_CORES):
        oc = res.results[c]["out"]    # [10, 480*len(TFS)]
        base = c * segs
        for t, tf in enumerate(TFS):
            gt = tf // J
            blk = oc[:, 480 * t:480 * t + 2 * gt]
            for s in (0, 1):
                sb = base + s * SEG_HALF + segbase[t]
                out[sb:sb + gt] = blk[:, s * gt:(s + 1) * gt].T
    return out


# revision 19
# speedup vs baseline: 3.2573x; 1.6510x over previous
"""Trainium2 Bass kernel for segment-reduce classifier.

Reference computation:
    local = relu(x @ Wloc.T)            # [L, 128]
    feats = local.reshape(-1, 30, 128).mean(1)   # [L/30, 128]
    out   = feats @ W.T                 # [L/30, 10]

Strategy v4 (8 NeuronCores, data-parallel on rows):
  - Per core xt [128, 75000] fp16: partitions 0-63 = rows[:75000].T ("A"),
    64-127 = rows[75000:].T ("B").  Within each outer tile (7680 rows =
    256 segments per stream) the host pre-permutes rows to tile-scope
    j-major (r = j*gt + g), so every on-chip access pattern is contiguous:
    mm1 streams flat 512-col chunks, relu reads flat PSUM, and mm2's per-j
    operand is two contiguous gt-element runs.
  - mm1: each [128, 1024] PSUM tile holds one A-chunk (bank 0, PE rows
    0-63) and one B-chunk (bank 1, rows 64-127); the two matmuls are
    adjacent in the queue and run concurrently on disjoint row quadrants.
    ppp bufs=3 keeps the PE filling tile k+1 while relus drain k/k-1.
  - relu PSUM fp32 -> SBUF fp16, one flat [p, 1024] instruction per pp
    tile, engines alternating Scalar(8) : Vector(7) (~1 elem/lane/cycle
    each on fp32 PSUM reads; this pass is the ~80us/core roofline).
  - mm2 (mean-pool + classifier fused): per outer tile, 30 accumulating
    512-col matmuls (rhs = rl[:, h, j*gt:(j+1)*gt]) into a per-tile PSUM
    accumulator [10, 2*gt] at column-group 32*(t%4); pooling is free PSUM
    accumulation, 1/30 folded into W.  js are deferred and dripped two
    per slot from the two oldest pending tiles, so adjacent mm2s use
    different column groups and pair up on the PE.
  - Per tile: one [10, 2*gt] copy PSUM->SBUF (alternating engines) + DMA.
"""

import numpy as np

import concourse.bacc as bacc
import concourse.bass as bass
import concourse.tile as tile
from concourse import mybir
from concourse.bass_utils import run_bass_kernel_spmd

# Problem constants (hardcoded per harness contract)
L, D_IN, D_ENC, C, J = 1200000, 64, 128, 10, 30
N_CORES = 8
R = L // N_CORES          # rows per core = 150000
HALF = R // 2             # 75000 rows per quadrant stream
SEG_HALF = HALF // J      # 2500 segments per stream
TFS = [7680] * 9 + [5880]  # rows per outer tile per stream
CH = 512                  # rows per PSUM bank chunk

_CACHE = {}


def _build_kernel():
    nc = bacc.Bacc("TRN2", target_bir_lowering=False, debug=False,
                   num_devices=N_CORES)
    f32, f16 = mybir.dt.float32, mybir.dt.float16

    xt_d = nc.dram_tensor("xt", [128, HALF], f16, kind="ExternalInput")
    w1_d = nc.dram_tensor("w1", [128, D_ENC], f16, kind="ExternalInput")
    w2_d = nc.dram_tensor("w2", [128, C], f16, kind="ExternalInput")
    # tile t at cols [512t, 512t + 2*gt); half h at h*gt + seg
    out_d = nc.dram_tensor("out", [C, 512 * len(TFS)], f32,
                           kind="ExternalOutput")

    with tile.TileContext(nc) as tc:
        with (
            tc.tile_pool(name="consts", bufs=1) as consts,
            tc.tile_pool(name="xin", bufs=3) as xin,
            tc.tile_pool(name="rlp", bufs=3) as rlp,
            tc.tile_pool(name="stgp", bufs=2) as stgp,
            tc.tile_pool(name="ppp", bufs=3, space="PSUM") as ppp,
            tc.tile_pool(name="accp", bufs=2, space="PSUM") as accp,
        ):
            w1 = consts.tile([128, D_ENC], f16)
            nc.sync.dma_start(w1[:], w1_d[:])
            w2 = consts.tile([128, C], f16)
            nc.sync.dma_start(w2[:], w2_d[:])

            pp_i = 0       # global relu counter (engine pattern, 8A:7D)
            pends = []     # lists of deferred mm2 emitters, one per tile
            rr = [0]       # round-robin cursor over pends

            def make_mm2(rl_t, tf, t):
                g = t % 4
                gt = tf // J
                rlh = rl_t.rearrange("p (h q) -> p h q", h=2)
                box = {}

                def emit(j):
                    if j == 0:
                        box["acc"] = accp.tile([128, 512], f32, tag="acc",
                                               name="acc")
                    acc = box["acc"]
                    av = acc[32 * g:32 * g + C, 0:2 * gt].rearrange(
                        "p (h q) -> p h q", h=2)
                    nc.tensor.matmul(av, w2[:],
                                     rlh[:, :, j * gt:(j + 1) * gt],
                                     start=(j == 0), stop=(j == J - 1),
                                     tile_position=(0, 32 * g))
                    if j == J - 1:
                        stg = stgp.tile([128, 512], f32, tag="stg",
                                        name="stg")
                        sl = slice(32 * g, 32 * g + C)
                        if t % 2 == 0:
                            nc.scalar.copy(stg[sl, 0:2 * gt],
                                           acc[sl, 0:2 * gt])
                        else:
                            nc.vector.tensor_scalar_add(
                                stg[sl, 0:2 * gt], acc[sl, 0:2 * gt], 0.0)
                        nc.sync.dma_start(out_d[:, 512 * t:512 * t + 2 * gt],
                                          stg[sl, 0:2 * gt])
                    return
                return emit

            def drip(n):
                for _ in range(n):
                    live = [d for d in pends if d]
                    if not live:
                        return
                    d = live[rr[0] % len(live)]
                    rr[0] += 1
                    d.pop(0)()
                while pends and not pends[0]:
                    pends.pop(0)

            col0 = 0
            for t, tf in enumerate(TFS):
                xt = xin.tile([128, 7680], f16, tag="xt", name="xt")
                nc.sync.dma_start(xt[:, 0:tf], xt_d[:, col0:col0 + tf])

                rl_t = rlp.tile([128, 15360], f16, tag="rl", name="rl")
                rlo = rl_t.rearrange("p (h q) -> p h q", h=2)

                n_pp = (tf + CH - 1) // CH
                pend_total = sum(len(d) for d in pends)
                per_slot = -(-pend_total // (2 * n_pp)) if pend_total else 0

                for k in range(n_pp):
                    o = k * CH
                    w = min(CH, tf - o)
                    drip(per_slot)
                    ppt = ppp.tile([128, 1024], f32, tag="pp", name="pp")
                    nc.tensor.matmul(ppt[:, 0:w], w1[0:64, :],
                                     xt[0:64, o:o + w],
                                     tile_position=(0, 0))
                    nc.tensor.matmul(ppt[:, 512:512 + w], w1[64:128, :],
                                     xt[64:128, o:o + w],
                                     tile_position=(64, 0))
                    src = ppt[:, 0:1024].rearrange(
                        "p (h k) -> p h k", h=2)[:, :, 0:w]
                    dst = rlo[:, :, o:o + w]
                    if (pp_i % 15) % 2 == 0:
                        nc.scalar.activation(
                            dst, src, mybir.ActivationFunctionType.Relu)
                    else:
                        nc.vector.tensor_scalar_max(dst, src, 0.0)
                    pp_i += 1

                pends.append([])
                em = make_mm2(rl_t, tf, t)
                pends[-1].extend([lambda j=j, em=em: em(j)
                                  for j in range(J)])
                col0 += tf

            drip(10 ** 6)

    nc.compile()
    return nc


def kernel(x: np.ndarray, Wloc: np.ndarray, W: np.ndarray) -> np.ndarray:
    if "nc" not in _CACHE:
        _CACHE["nc"] = _build_kernel()
    nc = _CACHE["nc"]

    x = np.asarray(x, dtype=np.float32)
    # per-core halves; tile-scope j-major permute; transpose to [64, HALF]
    xh = x.reshape(N_CORES * 2, HALF, D_IN)
    parts = []
    o = 0
    for tf in TFS:
        gt = tf // J
        blk = xh[:, o:o + tf].reshape(-1, gt, J, D_IN).transpose(0, 2, 1, 3)
        parts.append(blk.reshape(N_CORES * 2, tf, D_IN))
        o += tf
    xp = np.concatenate(parts, axis=1)
    xp = xp.reshape(N_CORES, 2, HALF, D_IN).transpose(0, 1, 3, 2)
    xp = np.ascontiguousarray(xp, dtype=np.float16).reshape(N_CORES, 128, HALF)

    w1 = np.ascontiguousarray(
        np.concatenate([Wloc.T, Wloc.T], axis=0), dtype=np.float16)  # [128,128]
    w2 = np.ascontiguousarray((W / float(J)).T, dtype=np.float16)    # [128,10]

    in_maps = [{"xt": xp[c], "w1": w1, "w2": w2} for c in range(N_CORES)]
    res = run_bass_kernel_spmd(nc, in_maps, core_ids=list(range(N_CORES)))
    _CACHE["exec_time_ns"] = res.exec_time_ns
    _CACHE["trace"] = res.instructions_and_trace

    out = np.empty((L // J, C), dtype=np.float32)
    segs = L // J // N_CORES          # 5000
    segbase = np.cumsum([0] + [tf // J for tf in TFS])
    for c in range(N_CORES):
        oc = res.results[c]["out"]    # [10, 512*len(TFS)]
        base = c * segs
        for t, tf in enumerate(TFS):
            gt = tf // J
            blk = oc[:, 512 * t:512 * t + 2 * gt]
            for s in (0, 1):
                sb = base + s * SEG_HALF + segbase[t]
                out[sb:sb + gt] = blk[:, s * gt:(s + 1) * gt].T
    return out
